# revision 7
# baseline (speedup 1.0000x reference)
"""Trainium2 kernel for nn_ClusteringLayer (vq_codebook).

Problem: x (1, 131072, 256) f32, cluster_centers (1024, 256) f32.
For each cluster k: find argmin_n ||x[n] - c[k]||^2, return that x row.
Output: (1, 1024, 256) f32.

v5 strategy (8 cores, x sharded along n, centers replicated):
  argmin_n d2[n,k] == argmax_n s[n,k],  s = 2*x.c - |x|^2.
  Host sorts points by |x|^2 (so |x|^2 is nearly constant inside each
  contiguous 2048-point group) and quantizes to fp8 e4m3.
  Device per core (16384 points x 1024 clusters, 64 tiles of
  [128 clusters, 2048 points] PSUM f32):
    - fp8 DoubleRow matmuls (256-deep contraction, 0.5 cyc/row) score
      the tile in 4 x 512-free matmuls.
    - 29 tiles: ONE DVE tensor_reduce f32 PSUM -> exact group max of
      s~ = 2*x.c in fp8 space (~2.28 us each).
    - 35 tiles: ONE Act exp-accumulate: A = sum_n exp(s~ - B[k,g]) with
      per-(cluster,group) bias B predicted from ||2c_k||*max||x||_g
      (~1.92 us each).  Host recovers max via log-sum-exp bounds:
      log(A)+B in [m~, m~ + ln 2048]; A==0/inf falls back to a forced
      rescore of that (cluster, group) - unconditionally safe.
    The two engines write accumulators into SEPARATE tiles (bmaxD /
    bmaxA) - a shared tile serializes the engines through WAW deps.
  Host recovery per cluster: interval bounds from [bm_lb, bm_ub] and the
  group's [x2min, x2max] widened by THETA (covers fp8 quantization
  noise, measured max |ds| = 7.49 on this input) select groups to
  rescore exactly (f32 gemm + f64 refine, first-original-index
  tiebreak).
"""

import os
import sys

for _p in ("/opt/trn_rl_repo",):
    if os.path.isdir(_p) and _p not in sys.path:
        sys.path.append(_p)

import numpy as np
import ml_dtypes

import concourse.bass as bass
import concourse.bacc as bacc
import concourse.mybir as mybir
import concourse.tile as tile

NCORES = 8
N = 131072
F = 256
K = 1024
SH = N // NCORES            # 16384 points per core
GRP = 2048                  # group size (device reduction + host bounds)
NG = SH // GRP              # 8 groups per core
NGRP = NCORES * NG          # 64 groups total
KT = K // 128               # 8 cluster tiles
THETA = 16.0                # covers 2*max fp8 score noise (measured 7.49)
TOPM = 32                   # fp32->fp64 refine width per (cluster, group)

BETA = 1.0                  # exp scale
BPRED = 0.2513              # B[k,g] = BPRED * ||2c_k|| * sqrt(x2max_g)
LOG_GRP = float(np.log(GRP))
EXTRA_ACT_GS = (0, 3, 6)    # groups with a 5th Act tile (offset 0)

E4 = ml_dtypes.float8_e4m3fn
MAXOP = mybir.AluOpType.max


def is_act_tile(g, kt):
    """Whether tile (g, kt) is reduced on Act (exp-accumulate)."""
    o = (kt - g) % KT
    return (o % 2 == 1) or (o == 0 and g in EXTRA_ACT_GS)


def build_nc():
    nc = bacc.Bacc("TRN2", target_bir_lowering=False, debug=False,
                   num_devices=NCORES)

    xt = nc.dram_tensor("xt", [2, 128, SH], mybir.dt.float8e4,
                        kind="ExternalInput")
    ct2 = nc.dram_tensor("ct2", [128, 2, K], mybir.dt.float8e4,
                         kind="ExternalInput")
    biasn = nc.dram_tensor("biasn", [128, NG * KT], mybir.dt.float32,
                           kind="ExternalInput")
    bmaxD_d = nc.dram_tensor("bmaxD", [128, NG * KT], mybir.dt.float32,
                             kind="ExternalOutput")
    bmaxA_d = nc.dram_tensor("bmaxA", [128, NG * KT], mybir.dt.float32,
                             kind="ExternalOutput")

    DR = mybir.MatmulPerfMode.DoubleRow

    with tile.TileContext(nc) as tc:
        with (
            tc.tile_pool(name="consts", bufs=1) as cpool,
            tc.tile_pool(name="xtp", bufs=3) as xpool,
            tc.tile_pool(name="psum", bufs=2, space="PSUM") as ppool,
        ):
            # --- constants: start DMAs first so they run at t=0 ---
            ct2_t = cpool.tile([128, 2, K], mybir.dt.float8e4, tag="ct")
            for h in range(2):
                nc.sync.dma_start(ct2_t[:, h, :], ct2[:, h, :])
            biasn_t = cpool.tile([128, NG * KT], mybir.dt.float32,
                                 tag="biasn")
            nc.sync.dma_start(biasn_t[:], biasn[:, :])
            bmaxD_t = cpool.tile([128, NG * KT], mybir.dt.float32,
                                 tag="bmaxD")
            bmaxA_t = cpool.tile([128, NG * KT], mybir.dt.float32,
                                 tag="bmaxA")

            # persistent junk output for Act exp (values never read back)
            junkA = cpool.tile([128, GRP], mybir.dt.float32, tag="ja")

            # --- warmup: PE pstate ramp + Exp act table load ---
            warm_w = cpool.tile([128, 2, 128], mybir.dt.float8e4, tag="warmw")
            warm_x = cpool.tile([128, 2, 512], mybir.dt.float8e4, tag="warmx")
            nc.gpsimd.memset(warm_w[:], 0.0)
            nc.gpsimd.memset(warm_x[:], 0.0)
            warm_f = cpool.tile([128, 16], mybir.dt.float32, tag="warmf")
            nc.gpsimd.memset(warm_f[:], 0.0)
            warm_h = cpool.tile([128, 16], mybir.dt.float32, tag="warmh")
            nc.scalar.activation(warm_h[:], warm_f[:],
                                 mybir.ActivationFunctionType.Exp)
            warm_ps = ppool.tile([128, GRP], mybir.dt.float32, tag="ps",
                                 name="warmps")
            for i in range(8):
                nc.tensor.matmul(warm_ps[:, (i % 4) * 512:(i % 4) * 512 + 512],
                                 lhsT=warm_w[:], rhs=warm_x[:],
                                 start=True, stop=True, perf_mode=DR)

            for g in range(NG):
                xs = xpool.tile([128, 2, GRP], mybir.dt.float8e4, tag="xs")
                base = g * GRP
                for p in range(2):
                    for hh in range(2):
                        nc.sync.dma_start(
                            xs[:, p, hh * 1024:(hh + 1) * 1024],
                            xt[p, :, base + hh * 1024:base + (hh + 1) * 1024])

                for kt in range(KT):
                    ps = ppool.tile([128, GRP], mybir.dt.float32, tag="ps")
                    lhsT = ct2_t[:, :, kt * 128:(kt + 1) * 128]
                    for j in range(4):
                        nc.tensor.matmul(
                            ps[:, j * 512:(j + 1) * 512],
                            lhsT=lhsT,
                            rhs=xs[:, :, j * 512:(j + 1) * 512],
                            start=True, stop=True, perf_mode=DR)
                    col = g * KT + kt
                    if is_act_tile(g, kt):
                        # Act: A = sum exp(beta*s + bias), bias = -beta*B
                        nc.scalar.activation(
                            junkA[:], ps[:],
                            mybir.ActivationFunctionType.Exp,
                            bias=biasn_t[:, col:col + 1], scale=BETA,
                            accum_out=bmaxA_t[:, col:col + 1])
                    else:
                        # DVE: exact tile max in one instruction
                        nc.vector.tensor_reduce(
                            out=bmaxD_t[:, col:col + 1], in_=ps[:],
                            axis=mybir.AxisListType.X, op=MAXOP)

            nc.sync.dma_start(bmaxD_d[:, :], bmaxD_t[:])
            nc.sync.dma_start(bmaxA_d[:, :], bmaxA_t[:])

    nc.compile()
    return nc


def host_prep(x, cluster_centers):
    """Sort points by |x|^2; build per-core fp8 device inputs."""
    x0 = np.ascontiguousarray(x[0], dtype=np.float32)        # (N, F)
    C = np.ascontiguousarray(cluster_centers, dtype=np.float32)
    x2 = np.einsum('nf,nf->n', x0.astype(np.float64),
                   x0.astype(np.float64))
    order = np.argsort(x2, kind="stable").astype(np.int64)
    xs_all = x0[order]
    x2s = x2[order]
    ct2_np = np.ascontiguousarray(
        (2.0 * C).T.astype(E4).reshape(2, 128, K).transpose(1, 0, 2))

    # per-(cluster, group) exp bias predictions
    cn = np.linalg.norm(2.0 * C.astype(np.float64), axis=1)   # (K,)
    gmax = np.sqrt(x2s.reshape(NGRP, GRP).max(axis=1))        # (NGRP,)
    B = BPRED * cn[:, None] * gmax[None, :]                   # (K, NGRP)

    in_maps = []
    for c in range(NCORES):
        xs = xs_all[c * SH:(c + 1) * SH]
        xt_np = np.ascontiguousarray(xs.T.astype(E4)).reshape(2, 128, SH)
        # biasn[p, g*KT+kt] = -BETA * B[kt*128+p, c*NG+g]
        Bc = B[:, c * NG:(c + 1) * NG]                        # (K, NG)
        bias_np = np.ascontiguousarray(
            (-BETA * Bc).reshape(KT, 128, NG).transpose(1, 2, 0)
            .reshape(128, NG * KT).astype(np.float32))
        in_maps.append({"xt": xt_np, "ct2": ct2_np, "biasn": bias_np})
    return in_maps, x0, C, order, xs_all, x2s


def host_combine(results, x0, C, order, xs_all, x2s):
    """Exact argmin recovery from per-group maxima / LSE of 2*dot.

    results: per-core dicts with 'bmaxD' and 'bmaxA' [128, NG*KT] f32.
    """
    x64s = xs_all.astype(np.float64)
    C64 = C.astype(np.float64)
    x2s_32 = x2s.astype(np.float32)

    # [128, NG*KT] col = g*KT + kt; k = kt*128 + p  ->  (K, NGRP)
    def to_kg(arrs):
        out = np.empty((K, NGRP), dtype=np.float32)
        for c in range(NCORES):
            a = np.asarray(arrs[c]).reshape(128, NG, KT)
            out[:, c * NG:(c + 1) * NG] = a.transpose(2, 0, 1).reshape(K, NG)
        return out

    bmD = to_kg([r["bmaxD"] for r in results])
    bmA = to_kg([r["bmaxA"] for r in results])

    # recompute the bias matrix (same as host_prep)
    cn = np.linalg.norm(2.0 * C.astype(np.float64), axis=1)
    gmax = np.sqrt(x2s.reshape(NGRP, GRP).max(axis=1))
    B = (BPRED * cn[:, None] * gmax[None, :]).astype(np.float64)

    # which (k, p) cells came from the Act (exp) path
    kt_of_k = np.arange(K) // 128                           # (K,)
    g_of_p = np.arange(NGRP) % NG                           # (NGRP,)
    off = (kt_of_k[:, None] - g_of_p[None, :]) % KT         # (K, NGRP)
    exp_mask = (off % 2 == 1) | ((off == 0)
                                 & np.isin(g_of_p, EXTRA_ACT_GS)[None, :])

    bm_ub = bmD.astype(np.float64)
    bm_lb = bmD.astype(np.float64)
    A = bmA.astype(np.float64)[exp_mask]
    with np.errstate(divide="ignore", over="ignore"):
        lse = np.log(A) / BETA + B[exp_mask]
    bad = ~np.isfinite(lse)
    ub = lse.copy()
    lb = lse - LOG_GRP / BETA
    ub[bad] = 1e30
    lb[bad] = -1e30
    bm_ub[exp_mask] = ub
    bm_lb[exp_mask] = lb

    gb = np.arange(NGRP) * GRP
    x2min = x2s[gb]
    x2max = x2s[gb + GRP - 1]

    ubs = bm_ub - x2min[None, :]
    lbs = bm_lb - x2max[None, :]
    win_lb = lbs.max(axis=1)
    flags = ubs >= (win_lb[:, None] - THETA)       # (K, NGRP)

    all_srt = []
    all_k = []
    for p in range(NGRP):
        ks = np.nonzero(flags[:, p])[0]
        if ks.size == 0:
            continue
        base = p * GRP
        pts = xs_all[base:base + GRP]
        d32 = x2s_32[base:base + GRP, None] - 2.0 * (pts @ C[ks].T)
        m = min(TOPM, GRP - 1)
        part = np.argpartition(d32, m, axis=0)[:m]      # (m, nk)
        all_srt.append(base + part.T)                   # (nk, m)
        all_k.append(ks)
    all_srt = np.concatenate(all_srt, axis=0)           # (P, m)
    all_k = np.concatenate(all_k, axis=0)               # (P,)

    ptsel = x64s[all_srt]                               # (P, m, F)
    dv = x2s[all_srt] - 2.0 * np.einsum('pmf,pf->pm', ptsel, C64[all_k])
    ids = order[all_srt]                                # (P, m)
    mrow = dv.min(axis=1, keepdims=True)
    idm = np.where(dv == mrow, ids, np.int64(2) ** 62)
    row_id = idm.min(axis=1)                            # (P,)
    row_dv = mrow[:, 0]                                 # (P,)

    o = np.lexsort((row_id, row_dv, all_k))
    ks_sorted = all_k[o]
    first = np.ones(len(o), dtype=bool)
    first[1:] = ks_sorted[1:] != ks_sorted[:-1]
    sel = o[first]
    best_idx = np.zeros(K, dtype=np.int64)
    best_idx[all_k[sel]] = row_id[sel]
    assert np.all(np.bincount(all_k, minlength=K) > 0), "uncovered cluster"

    return x0[best_idx][None].astype(np.float32)


_NC_CACHE = {}


def kernel(x, cluster_centers):
    from concourse.bass_utils import run_bass_kernel_spmd

    if "nc" not in _NC_CACHE:
        _NC_CACHE["nc"] = build_nc()
    nc = _NC_CACHE["nc"]

    in_maps, x0, C, order, xs_all, x2s = host_prep(x, cluster_centers)
    res = run_bass_kernel_spmd(nc, in_maps, list(range(NCORES)))
    return host_combine([res.results[c] for c in range(NCORES)],
                        x0, C, order, xs_all, x2s)


# revision 8
# speedup vs baseline: 10681.4488x; 10681.4488x over previous
"""Trainium2 kernel for nn_ClusteringLayer (vq_codebook).

Problem: x (1, 131072, 256) f32, cluster_centers (1024, 256) f32.
For each cluster k: find argmin_n ||x[n] - c[k]||^2, return that x row.
Output: (1, 1024, 256) f32.

v6 strategy (8 cores, x sharded along n, centers replicated):
  argmin_n d2[n,k] == argmax_n s[n,k],  s = 2*x.c - |x|^2.
  Host sorts points by |x|^2 (so |x|^2 is nearly constant inside each
  contiguous 1024-point group) and quantizes to fp8 e4m3.
  Device per core (16384 points x 1024 clusters, 128 tiles of
  [128 clusters, 1024 points] PSUM f32, 4 PSUM buffers -> pipeline
  depth 4 so the matmul fill + semaphore hops hide under the
  reductions; at depth 2 the fill sits on the critical path):
    - fp8 DoubleRow matmuls (256-deep contraction, 0.5 cyc/row) score
      each tile in 2 x 512-free matmuls.
    - 66 tiles: ONE DVE tensor_reduce f32 PSUM -> exact group max of
      s~ = 2*x.c in fp8 space (~1.2 us each).
    - 62 tiles: ONE Act exp-accumulate: A = sum_n exp(s~ - B[k,g]) with
      per-(cluster,group) bias B predicted from ||2c_k||*max||x||_g
      (~1.3 us each).  Host recovers max via log-sum-exp bounds:
      log(A)+B in [m~, m~ + ln 1024]; A==0/inf falls back to a forced
      rescore of that (cluster, group) - unconditionally safe.
    The two engines write accumulators into SEPARATE tiles (bmaxD /
    bmaxA) - a shared tile serializes the engines through WAW deps.
  Host recovery per cluster: interval bounds from [bm_lb, bm_ub] and the
  group's [x2min, x2max] widened by THETA (covers fp8 quantization
  noise, measured max |ds| = 7.49 on this input) select groups to
  rescore exactly (f32 gemm + f64 refine, first-original-index
  tiebreak).
"""

import os
import sys

for _p in ("/opt/trn_rl_repo",):
    if os.path.isdir(_p) and _p not in sys.path:
        sys.path.append(_p)

import numpy as np
import ml_dtypes

import concourse.bass as bass
import concourse.bacc as bacc
import concourse.mybir as mybir
import concourse.tile as tile

NCORES = 8
N = 131072
F = 256
K = 1024
SH = N // NCORES            # 16384 points per core
GRP = 1024                  # group size (device reduction + host bounds)
NG = SH // GRP              # 16 groups per core
NGRP = NCORES * NG          # 128 groups total
KT = K // 128               # 8 cluster tiles
THETA = 16.0                # covers 2*max fp8 score noise (measured 7.49)
TOPM = 32                   # fp32->fp64 refine width per (cluster, group)

BETA = 1.0                  # exp scale
BPRED = 0.2513              # B[k,g] = BPRED * ||2c_k|| * sqrt(x2max_g)
LOG_GRP = float(np.log(GRP))
DVE_EXTRA_GS = (0, 8)       # groups where offset-1 tiles go to DVE (66:62)

E4 = ml_dtypes.float8_e4m3fn
MAXOP = mybir.AluOpType.max


def is_act_tile(g, kt):
    """Whether tile (g, kt) is reduced on Act (exp-accumulate)."""
    o = (kt - g) % KT
    return (o % 2 == 1) and not (o == 1 and g in DVE_EXTRA_GS)


def build_nc():
    nc = bacc.Bacc("TRN2", target_bir_lowering=False, debug=False,
                   num_devices=NCORES)

    xt = nc.dram_tensor("xt", [2, 128, SH], mybir.dt.float8e4,
                        kind="ExternalInput")
    ct2 = nc.dram_tensor("ct2", [128, 2, K], mybir.dt.float8e4,
                         kind="ExternalInput")
    biasn = nc.dram_tensor("biasn", [128, NG * KT], mybir.dt.float32,
                           kind="ExternalInput")
    bmaxD_d = nc.dram_tensor("bmaxD", [128, NG * KT], mybir.dt.float32,
                             kind="ExternalOutput")
    bmaxA_d = nc.dram_tensor("bmaxA", [128, NG * KT], mybir.dt.float32,
                             kind="ExternalOutput")

    DR = mybir.MatmulPerfMode.DoubleRow

    with tile.TileContext(nc) as tc:
        with (
            tc.tile_pool(name="consts", bufs=1) as cpool,
            tc.tile_pool(name="xtp", bufs=4) as xpool,
            tc.tile_pool(name="psum", bufs=4, space="PSUM") as ppool,
        ):
            # --- constants: start DMAs first so they run at t=0 ---
            ct2_t = cpool.tile([128, 2, K], mybir.dt.float8e4, tag="ct")
            nc.sync.dma_start(ct2_t[:], ct2[:, :, :])
            biasn_t = cpool.tile([128, NG * KT], mybir.dt.float32,
                                 tag="biasn")
            nc.sync.dma_start(biasn_t[:], biasn[:, :])
            bmaxD_t = cpool.tile([128, NG * KT], mybir.dt.float32,
                                 tag="bmaxD")
            bmaxA_t = cpool.tile([128, NG * KT], mybir.dt.float32,
                                 tag="bmaxA")

            # persistent junk output for Act exp (values never read back)
            junkA = cpool.tile([128, GRP], mybir.dt.float32, tag="ja")

            # --- warmup: PE pstate ramp + Exp act table load ---
            warm_w = cpool.tile([128, 2, 128], mybir.dt.float8e4, tag="warmw")
            warm_x = cpool.tile([128, 2, 512], mybir.dt.float8e4, tag="warmx")
            nc.gpsimd.memset(warm_w[:], 0.0)
            nc.gpsimd.memset(warm_x[:], 0.0)
            warm_f = cpool.tile([128, 16], mybir.dt.float32, tag="warmf")
            nc.gpsimd.memset(warm_f[:], 0.0)
            warm_h = cpool.tile([128, 16], mybir.dt.float32, tag="warmh")
            nc.scalar.activation(warm_h[:], warm_f[:],
                                 mybir.ActivationFunctionType.Exp)
            warm_ps = ppool.tile([128, GRP], mybir.dt.float32, tag="ps",
                                 name="warmps")
            for i in range(8):
                nc.tensor.matmul(warm_ps[:, (i % 2) * 512:(i % 2) * 512 + 512],
                                 lhsT=warm_w[:], rhs=warm_x[:],
                                 start=True, stop=True, perf_mode=DR)

            for g in range(NG):
                xs = xpool.tile([128, 2, GRP], mybir.dt.float8e4, tag="xs")
                base = g * GRP
                for p in range(2):
                    nc.sync.dma_start(xs[:, p, :],
                                      xt[p, :, base:base + GRP])

                for kt in range(KT):
                    ps = ppool.tile([128, GRP], mybir.dt.float32, tag="ps")
                    lhsT = ct2_t[:, :, kt * 128:(kt + 1) * 128]
                    for j in range(2):
                        nc.tensor.matmul(
                            ps[:, j * 512:(j + 1) * 512],
                            lhsT=lhsT,
                            rhs=xs[:, :, j * 512:(j + 1) * 512],
                            start=True, stop=True, perf_mode=DR)
                    col = g * KT + kt
                    if is_act_tile(g, kt):
                        # Act: A = sum exp(beta*s + bias), bias = -beta*B
                        nc.scalar.activation(
                            junkA[:], ps[:],
                            mybir.ActivationFunctionType.Exp,
                            bias=biasn_t[:, col:col + 1], scale=BETA,
                            accum_out=bmaxA_t[:, col:col + 1])
                    else:
                        # DVE: exact tile max in one instruction
                        nc.vector.tensor_reduce(
                            out=bmaxD_t[:, col:col + 1], in_=ps[:],
                            axis=mybir.AxisListType.X, op=MAXOP)

            nc.sync.dma_start(bmaxD_d[:, :], bmaxD_t[:])
            nc.sync.dma_start(bmaxA_d[:, :], bmaxA_t[:])

    nc.compile()
    return nc


def host_prep(x, cluster_centers):
    """Sort points by |x|^2; build per-core fp8 device inputs."""
    x0 = np.ascontiguousarray(x[0], dtype=np.float32)        # (N, F)
    C = np.ascontiguousarray(cluster_centers, dtype=np.float32)
    x2 = np.einsum('nf,nf->n', x0.astype(np.float64),
                   x0.astype(np.float64))
    order = np.argsort(x2, kind="stable").astype(np.int64)
    xs_all = x0[order]
    x2s = x2[order]
    ct2_np = np.ascontiguousarray(
        (2.0 * C).T.astype(E4).reshape(2, 128, K).transpose(1, 0, 2))

    # per-(cluster, group) exp bias predictions
    cn = np.linalg.norm(2.0 * C.astype(np.float64), axis=1)   # (K,)
    gmax = np.sqrt(x2s.reshape(NGRP, GRP).max(axis=1))        # (NGRP,)
    B = BPRED * cn[:, None] * gmax[None, :]                   # (K, NGRP)

    in_maps = []
    for c in range(NCORES):
        xs = xs_all[c * SH:(c + 1) * SH]
        xt_np = np.ascontiguousarray(xs.T.astype(E4)).reshape(2, 128, SH)
        # biasn[p, g*KT+kt] = -BETA * B[kt*128+p, c*NG+g]
        Bc = B[:, c * NG:(c + 1) * NG]                        # (K, NG)
        bias_np = np.ascontiguousarray(
            (-BETA * Bc).reshape(KT, 128, NG).transpose(1, 2, 0)
            .reshape(128, NG * KT).astype(np.float32))
        in_maps.append({"xt": xt_np, "ct2": ct2_np, "biasn": bias_np})
    return in_maps, x0, C, order, xs_all, x2s


def host_combine(results, x0, C, order, xs_all, x2s):
    """Exact argmin recovery from per-group maxima / LSE of 2*dot.

    results: per-core dicts with 'bmaxD' and 'bmaxA' [128, NG*KT] f32.
    """
    x64s = xs_all.astype(np.float64)
    C64 = C.astype(np.float64)
    x2s_32 = x2s.astype(np.float32)

    # [128, NG*KT] col = g*KT + kt; k = kt*128 + p  ->  (K, NGRP)
    def to_kg(arrs):
        out = np.empty((K, NGRP), dtype=np.float32)
        for c in range(NCORES):
            a = np.asarray(arrs[c]).reshape(128, NG, KT)
            out[:, c * NG:(c + 1) * NG] = a.transpose(2, 0, 1).reshape(K, NG)
        return out

    bmD = to_kg([r["bmaxD"] for r in results])
    bmA = to_kg([r["bmaxA"] for r in results])

    # recompute the bias matrix (same as host_prep)
    cn = np.linalg.norm(2.0 * C.astype(np.float64), axis=1)
    gmax = np.sqrt(x2s.reshape(NGRP, GRP).max(axis=1))
    B = (BPRED * cn[:, None] * gmax[None, :]).astype(np.float64)

    # which (k, p) cells came from the Act (exp) path
    kt_of_k = np.arange(K) // 128                           # (K,)
    g_of_p = np.arange(NGRP) % NG                           # (NGRP,)
    off = (kt_of_k[:, None] - g_of_p[None, :]) % KT         # (K, NGRP)
    exp_mask = (off % 2 == 1) & ~((off == 1)
                                  & np.isin(g_of_p, DVE_EXTRA_GS)[None, :])

    bm_ub = bmD.astype(np.float64)
    bm_lb = bmD.astype(np.float64)
    A = bmA.astype(np.float64)[exp_mask]
    with np.errstate(divide="ignore", over="ignore"):
        lse = np.log(A) / BETA + B[exp_mask]
    bad = ~np.isfinite(lse)
    ub = lse.copy()
    lb = lse - LOG_GRP / BETA
    ub[bad] = 1e30
    lb[bad] = -1e30
    bm_ub[exp_mask] = ub
    bm_lb[exp_mask] = lb

    gb = np.arange(NGRP) * GRP
    x2min = x2s[gb]
    x2max = x2s[gb + GRP - 1]

    ubs = bm_ub - x2min[None, :]
    lbs = bm_lb - x2max[None, :]
    win_lb = lbs.max(axis=1)
    flags = ubs >= (win_lb[:, None] - THETA)       # (K, NGRP)

    all_srt = []
    all_k = []
    for p in range(NGRP):
        ks = np.nonzero(flags[:, p])[0]
        if ks.size == 0:
            continue
        base = p * GRP
        pts = xs_all[base:base + GRP]
        d32 = x2s_32[base:base + GRP, None] - 2.0 * (pts @ C[ks].T)
        m = min(TOPM, GRP - 1)
        part = np.argpartition(d32, m, axis=0)[:m]      # (m, nk)
        all_srt.append(base + part.T)                   # (nk, m)
        all_k.append(ks)
    all_srt = np.concatenate(all_srt, axis=0)           # (P, m)
    all_k = np.concatenate(all_k, axis=0)               # (P,)

    ptsel = x64s[all_srt]                               # (P, m, F)
    dv = x2s[all_srt] - 2.0 * np.einsum('pmf,pf->pm', ptsel, C64[all_k])
    ids = order[all_srt]                                # (P, m)
    mrow = dv.min(axis=1, keepdims=True)
    idm = np.where(dv == mrow, ids, np.int64(2) ** 62)
    row_id = idm.min(axis=1)                            # (P,)
    row_dv = mrow[:, 0]                                 # (P,)

    o = np.lexsort((row_id, row_dv, all_k))
    ks_sorted = all_k[o]
    first = np.ones(len(o), dtype=bool)
    first[1:] = ks_sorted[1:] != ks_sorted[:-1]
    sel = o[first]
    best_idx = np.zeros(K, dtype=np.int64)
    best_idx[all_k[sel]] = row_id[sel]
    assert np.all(np.bincount(all_k, minlength=K) > 0), "uncovered cluster"

    return x0[best_idx][None].astype(np.float32)


_NC_CACHE = {}


def kernel(x, cluster_centers):
    from concourse.bass_utils import run_bass_kernel_spmd

    if "nc" not in _NC_CACHE:
        _NC_CACHE["nc"] = build_nc()
    nc = _NC_CACHE["nc"]

    in_maps, x0, C, order, xs_all, x2s = host_prep(x, cluster_centers)
    res = run_bass_kernel_spmd(nc, in_maps, list(range(NCORES)))
    return host_combine([res.results[c] for c in range(NCORES)],
                        x0, C, order, xs_all, x2s)
